# revision 12
# baseline (speedup 1.0000x reference)
"""Trainium2 Bass kernel for nn_DESMO (SINDy/POD reconstruction).

Math (per reference):
  y0 = phi[0] * POD[:,0],  y1 = phi[1] * POD[:,1]            (n,)
  out(m, n) = W^T @ F
    F (12, n) = [1, y0, y0^2, y0*y1, sin(w0*y0), cos(w1*y0), tanh(w2*y0),
                 y1, y1^2, sin(w3*y1), cos(w4*y1), tanh(w5*y1)]
    W (12, m) = [c0*z0, c1*z1, c3*z3, c4*z4, sc0*zsin0, cc0*zcos0, tc0*ztanh0,
                 c2*z2, c5*z5, sc1*zsin1, cc1*zcos1, tc1*ztanh1]
  latent (n, 2) = [y0, y1]
  z_values passthrough.

Sharding: n (spatial) split evenly across 8 NeuronCores; z rows / coefs
replicated; no cross-core communication.
"""

import numpy as np

from concourse import bacc, bass, mybir, tile
from concourse.bass_utils import run_bass_kernel_spmd

F32 = mybir.dt.float32
F32R = mybir.dt.float32r

N_CORES = 8
N, M, R = 100000, 1000, 2
NLOC = N // N_CORES          # 12500 spatial points per core
L = 98                       # free-dim elems per partition chunk
NL = 128 * L                 # 12544 = padded local n
NF = 448                     # matmul free-dim chunk (fits one PSUM bank)
QW = NL // 4                 # 3136 columns per PE row-group quarter
NCHUNK = QW // NF            # 7 chunks per quarter

# m tiles: 7 x 128 + 104
M_TILES = [(i * 128, min(128, M - i * 128)) for i in range((M + 127) // 128)]

_COMPILED = None


def _build():
    nc = bacc.Bacc(
        "TRN2",
        target_bir_lowering=False,
        debug=False,
        enable_asserts=True,
        num_devices=N_CORES,
    )

    phi_d = nc.dram_tensor("phi_l", [2, 128, L], F32, kind="ExternalInput")
    pod_d = nc.dram_tensor("pod_l", [2, 128, L], F32, kind="ExternalInput")
    # zcat/coef carry 4 replicas at partition offsets 0/32/64/96 so four
    # K=12 matmuls can run concurrently in the PE's four 32-row groups.
    zcat_d = nc.dram_tensor("zcat", [128, M], F32, kind="ExternalInput")
    coef_d = nc.dram_tensor("coef", [128, 1], F32, kind="ExternalInput")
    om_d = nc.dram_tensor("om", [128, 6], F32, kind="ExternalInput")

    out_d = nc.dram_tensor("out_l", [M, NLOC], F32, kind="ExternalOutput")
    lat_d = nc.dram_tensor("lat_l", [128, 2 * L], F32, kind="ExternalOutput")

    HALF_PI = float(np.pi / 2.0)
    Sin = mybir.ActivationFunctionType.Sin
    Tanh = mybir.ActivationFunctionType.Tanh

    with tile.TileContext(nc) as tc:
        with (
            tc.tile_pool(name="const", bufs=1) as constp,
            tc.tile_pool(name="feat", bufs=1) as featp,
            tc.tile_pool(name="fbig", bufs=1) as fbigp,
            tc.tile_pool(name="band", bufs=3) as bandp,
            tc.tile_pool(name="psum", bufs=8, space="PSUM") as psump,
        ):
            # ---- W = coef * zcat, replicated in 4 row groups ----
            zcat_t = constp.tile([128, M], F32, tag="zcat")
            coef_t = constp.tile([128, 1], F32, tag="coef")
            om_t = constp.tile([128, 6], F32, tag="om")
            w_t = constp.tile([128, M], F32R, tag="w")
            # ---- features, (128, L) p-major layout (inputs first) ----
            ph0 = featp.tile([128, L], F32, tag="ph0")
            ph1 = featp.tile([128, L], F32, tag="ph1")
            po0 = featp.tile([128, L], F32, tag="po0")
            po1 = featp.tile([128, L], F32, tag="po1")
            nc.sync.dma_start(ph0[:], phi_d[0])
            nc.sync.dma_start(po0[:], pod_d[0])
            nc.sync.dma_start(ph1[:], phi_d[1])
            nc.sync.dma_start(po1[:], pod_d[1])
            nc.sync.dma_start(zcat_t[:], zcat_d[:])
            nc.sync.dma_start(coef_t[:], coef_d[:])
            nc.sync.dma_start(om_t[:], om_d[:])
            nc.vector.tensor_scalar_mul(w_t[:], zcat_t[:], coef_t[:, 0:1])

            y0 = featp.tile([128, L], F32, tag="y0")
            y1 = featp.tile([128, L], F32, tag="y1")
            nc.vector.tensor_mul(y0[:], ph0[:], po0[:])
            nc.vector.tensor_mul(y1[:], ph1[:], po1[:])

            # latent output: interleave y0/y1 -> (n, 2) layout
            il = featp.tile([128, 2 * L], F32, tag="il")
            ilv = il[:].rearrange("p (j s) -> p j s", s=2)
            nc.vector.tensor_copy(ilv[:, :, 0], y0[:])
            nc.vector.tensor_copy(ilv[:, :, 1], y1[:])
            nc.sync.dma_start(lat_d[:], il[:])

            names = [
                "y0sq", "y0y1", "s0", "c0", "t0",
                "y1sq", "s1", "c1", "t1",
            ]
            ft = {nm: featp.tile([128, L], F32R, tag=nm, name=nm) for nm in names}
            nc.vector.tensor_mul(ft["y0sq"][:], y0[:], y0[:])
            nc.vector.tensor_mul(ft["y1sq"][:], y1[:], y1[:])
            nc.vector.tensor_mul(ft["y0y1"][:], y0[:], y1[:])

            # Trig features. The Sin activation is only valid on [-pi, pi],
            # so range-reduce a = omega*y first: k = round(a/2pi) (fp32
            # magic-number rounding), r = a - k*2pi, then wrap with the sin/
            # cos shift back into [-pi, pi] (rotary recipe).
            TWO_PI = float(2.0 * np.pi)
            PI = float(np.pi)
            MAGIC = float(1.5 * 2.0**23)
            mul_op = mybir.AluOpType.mult
            add_op = mybir.AluOpType.add

            def trig(dst, y_t, om_col, shift, idx):
                a = featp.tile([128, L], F32, tag=f"tga{idx}", name=f"tga{idx}")
                kf = featp.tile([128, L], F32, tag=f"tgk{idx}", name=f"tgk{idx}")
                rr = featp.tile([128, L], F32, tag=f"tgr{idx}", name=f"tgr{idx}")
                nc.vector.tensor_scalar_mul(a[:], y_t[:], om_t[:, om_col : om_col + 1])
                nc.vector.tensor_scalar(
                    kf[:], a[:], 1.0 / TWO_PI, MAGIC, mul_op, add_op
                )
                nc.vector.tensor_scalar_add(kf[:], kf[:], -MAGIC)
                nc.vector.scalar_tensor_tensor(
                    rr[:], kf[:], -TWO_PI, a[:], mul_op, add_op
                )
                nc.vector.add_range_wrap(a[:], rr[:], shift, PI, TWO_PI)
                nc.scalar.activation(dst[:], a[:], Sin)

            trig(ft["s0"], y0, 0, 0.0, 0)
            trig(ft["c0"], y0, 1, HALF_PI, 1)
            trig(ft["s1"], y1, 3, 0.0, 2)
            trig(ft["c1"], y1, 4, HALF_PI, 3)
            nc.scalar.activation(ft["t0"][:], y0[:], Tanh, scale=om_t[:, 2:3])
            nc.scalar.activation(ft["t1"][:], y1[:], Tanh, scale=om_t[:, 5:6])

            # ---- F matrix (12, NL): row k <- reshape of (128, L) feature ----
            # F matrix split in 4 quarters along n, quarter q at
            # partitions [32q, 32q+12): fq[32q+k, j] = F[k, 3136q + j]
            fbig = fbigp.tile([128, QW], F32R, tag="fbig")
            ones_f = featp.tile([128, L], F32, tag="ones_f")
            ones_r = featp.tile([128, L], F32R, tag="ones_r")
            nc.vector.memset(ones_f[:], 1.0)
            nc.vector.tensor_copy(ones_r[:], ones_f[:])
            y0r = featp.tile([128, L], F32R, tag="y0r")
            y1r = featp.tile([128, L], F32R, tag="y1r")
            nc.vector.tensor_copy(y0r[:], y0[:])
            nc.vector.tensor_copy(y1r[:], y1[:])
            row_srcs = [
                ones_r,
                y0r, ft["y0sq"], ft["y0y1"], ft["s0"], ft["c0"], ft["t0"],
                y1r, ft["y1sq"], ft["s1"], ft["c1"], ft["t1"],
            ]
            # issue in feature-readiness order (cheap poly rows first, trig
            # last); quarter-0 pieces first so the first matmuls unblock
            # before quarters 1-3 land.
            ready_order = [0, 1, 7, 2, 8, 3, 6, 11, 4, 5, 9, 10]
            for k in ready_order:
                srct = row_srcs[k]
                nc.sync.dma_start(fbig[k : k + 1, :QW], srct[0:32, :])
            for k in ready_order:
                srct = row_srcs[k]
                nc.sync.dma_start(fbig[32 + k :: 32, :], srct[32:128, :])

            # ---- main loop: out(m, n) = W^T @ F ----
            # Four concurrent K=12 matmuls per chunk, one per PE row group
            # (tile_position auto-derived from the base partition 32q).
            for ti, (m0, mt) in enumerate(M_TILES):
                band = bandp.tile([128, NL], F32, tag="band")
                if ti == 0:
                    # pair-major: 2-way PE packing; each quarter-pair's 3.2MB
                    # streams out while the next pair computes
                    qc = [(q, c) for qp in range(2)
                          for c in range(NCHUNK) for q in (2 * qp, 2 * qp + 1)]
                else:
                    qc = [(q, c) for c in range(NCHUNK) for q in range(4)]
                for q, c in qc:
                    ps = psump.tile([128, NF], F32, tag="ps")
                    nc.tensor.matmul(
                        ps[:mt, :],
                        w_t[32 * q : 32 * q + 12, m0 : m0 + mt],
                        fbig[32 * q : 32 * q + 12, c * NF : (c + 1) * NF],
                        start=True,
                        stop=True,
                        tile_position=(32 * q, 0),
                    )
                    col = QW * q + NF * c
                    if (q + c) % 2 == 0:
                        nc.vector.tensor_copy(band[:mt, col : col + NF], ps[:mt, :])
                    else:
                        nc.scalar.copy(band[:mt, col : col + NF], ps[:mt, :])
                    if ti == 0 and c == NCHUNK - 1 and q % 2 == 1:
                        # quarters stream out in pairs as they complete
                        lo = QW * (q - 1)
                        hi = min(QW * (q + 1), NLOC)
                        nc.sync.dma_start(
                            out_d[m0 : m0 + mt, lo:hi], band[:mt, lo:hi]
                        )
                if ti > 0:
                    nc.sync.dma_start(
                        out_d[m0 : m0 + mt, :], band[:mt, :NLOC]
                    )

    nc.compile()
    return nc


def _get_compiled():
    global _COMPILED
    if _COMPILED is None:
        _COMPILED = _build()
    return _COMPILED


def _make_in_maps(inputs):
    phi = np.ascontiguousarray(np.asarray(inputs["phi"], dtype=np.float32))
    pod = np.asarray(inputs["POD_modes"], dtype=np.float32)
    podT = np.ascontiguousarray(pod.T)  # (2, N)
    c = np.asarray(inputs["c_coef"], dtype=np.float32)
    zv = np.asarray(inputs["z_values"], dtype=np.float32)
    zs = np.asarray(inputs["zsin"], dtype=np.float32)
    zc = np.asarray(inputs["zcos"], dtype=np.float32)
    zt = np.asarray(inputs["ztanh"], dtype=np.float32)
    sc = np.asarray(inputs["sin_coef"], dtype=np.float32)
    cc = np.asarray(inputs["cos_coef"], dtype=np.float32)
    tc_ = np.asarray(inputs["tanh_coef"], dtype=np.float32)
    om = np.asarray(inputs["omega"], dtype=np.float32)

    zcat12 = np.stack(
        [zv[0], zv[1], zv[3], zv[4], zs[0], zc[0], zt[0],
         zv[2], zv[5], zs[1], zc[1], zt[1]], axis=0
    ).astype(np.float32)
    coef12 = np.array(
        [c[0], c[1], c[3], c[4], sc[0], cc[0], tc_[0],
         c[2], c[5], sc[1], cc[1], tc_[1]], dtype=np.float32
    ).reshape(12, 1)
    zcat = np.zeros((128, M), np.float32)
    coef = np.zeros((128, 1), np.float32)
    for q in range(4):
        zcat[32 * q : 32 * q + 12] = zcat12
        coef[32 * q : 32 * q + 12] = coef12
    om_rep = np.ascontiguousarray(np.tile(om.reshape(1, 6), (128, 1)))

    in_maps = []
    for ci in range(N_CORES):
        lo = ci * NLOC
        phi_l = np.zeros((2, NL), np.float32)
        pod_l = np.zeros((2, NL), np.float32)
        phi_l[:, :NLOC] = phi[:, lo : lo + NLOC]
        pod_l[:, :NLOC] = podT[:, lo : lo + NLOC]
        in_maps.append(
            {
                "phi_l": phi_l.reshape(2, 128, L),
                "pod_l": pod_l.reshape(2, 128, L),
                "zcat": zcat,
                "coef": coef,
                "om": om_rep,
            }
        )
    return in_maps


def _run(inputs, trace=False):
    nc = _get_compiled()
    in_maps = _make_in_maps(inputs)
    res = run_bass_kernel_spmd(nc, in_maps, core_ids=list(range(N_CORES)), trace=trace)

    out = np.empty((M, N), np.float32)
    lat = np.empty((N, 2), np.float32)
    for ci in range(N_CORES):
        lo = ci * NLOC
        out[:, lo : lo + NLOC] = res.results[ci]["out_l"]
        lat[lo : lo + NLOC] = res.results[ci]["lat_l"].reshape(-1, 2)[:NLOC]
    zv = np.asarray(inputs["z_values"], dtype=np.float32)
    return (out, lat, zv), res


def kernel(**inputs):
    (out, lat, zv), _ = _run(inputs, trace=False)
    return out, lat, zv


# revision 13
# speedup vs baseline: 1.0036x; 1.0036x over previous
"""Trainium2 Bass kernel for nn_DESMO (SINDy/POD reconstruction).

Math (per reference):
  y0 = phi[0] * POD[:,0],  y1 = phi[1] * POD[:,1]            (n,)
  out(m, n) = W^T @ F
    F (12, n) = [1, y0, y0^2, y0*y1, sin(w0*y0), cos(w1*y0), tanh(w2*y0),
                 y1, y1^2, sin(w3*y1), cos(w4*y1), tanh(w5*y1)]
    W (12, m) = [c0*z0, c1*z1, c3*z3, c4*z4, sc0*zsin0, cc0*zcos0, tc0*ztanh0,
                 c2*z2, c5*z5, sc1*zsin1, cc1*zcos1, tc1*ztanh1]
  latent (n, 2) = [y0, y1]
  z_values passthrough.

Sharding: n (spatial) split evenly across 8 NeuronCores; z rows / coefs
replicated; no cross-core communication.
"""

import numpy as np

from concourse import bacc, bass, mybir, tile
from concourse.bass_utils import run_bass_kernel_spmd

F32 = mybir.dt.float32
F32R = mybir.dt.float32r

N_CORES = 8
N, M, R = 100000, 1000, 2
NLOC = N // N_CORES          # 12500 spatial points per core
L = 98                       # free-dim elems per partition chunk
NL = 128 * L                 # 12544 = padded local n
NF = 448                     # matmul free-dim chunk (fits one PSUM bank)
QW = NL // 4                 # 3136 columns per PE row-group quarter
NCHUNK = QW // NF            # 7 chunks per quarter

# m tiles: 7 x 128 + 104
M_TILES = [(i * 128, min(128, M - i * 128)) for i in range((M + 127) // 128)]

_COMPILED = None


def _build():
    nc = bacc.Bacc(
        "TRN2",
        target_bir_lowering=False,
        debug=False,
        enable_asserts=True,
        num_devices=N_CORES,
    )

    phi_d = nc.dram_tensor("phi_l", [2, 128, L], F32, kind="ExternalInput")
    pod_d = nc.dram_tensor("pod_l", [2, 128, L], F32, kind="ExternalInput")
    # zcat/coef carry 4 replicas at partition offsets 0/32/64/96 so four
    # K=12 matmuls can run concurrently in the PE's four 32-row groups.
    zcat_d = nc.dram_tensor("zcat", [128, M], F32, kind="ExternalInput")
    coef_d = nc.dram_tensor("coef", [128, 1], F32, kind="ExternalInput")
    om_d = nc.dram_tensor("om", [128, 6], F32, kind="ExternalInput")

    out_d = nc.dram_tensor("out_l", [M, NL], F32, kind="ExternalOutput")
    lat_d = nc.dram_tensor("lat_l", [128, 2 * L], F32, kind="ExternalOutput")

    HALF_PI = float(np.pi / 2.0)
    Sin = mybir.ActivationFunctionType.Sin
    Tanh = mybir.ActivationFunctionType.Tanh

    with tile.TileContext(nc) as tc:
        with (
            tc.tile_pool(name="const", bufs=1) as constp,
            tc.tile_pool(name="feat", bufs=1) as featp,
            tc.tile_pool(name="fbig", bufs=1) as fbigp,
            tc.tile_pool(name="band", bufs=3) as bandp,
            tc.tile_pool(name="psum", bufs=8, space="PSUM") as psump,
        ):
            # ---- W = coef * zcat, replicated in 4 row groups ----
            zcat_t = constp.tile([128, M], F32, tag="zcat")
            coef_t = constp.tile([128, 1], F32, tag="coef")
            om_t = constp.tile([128, 6], F32, tag="om")
            w_t = constp.tile([128, M], F32R, tag="w")
            # ---- features, (128, L) p-major layout (inputs first) ----
            ph0 = featp.tile([128, L], F32, tag="ph0")
            ph1 = featp.tile([128, L], F32, tag="ph1")
            po0 = featp.tile([128, L], F32, tag="po0")
            po1 = featp.tile([128, L], F32, tag="po1")
            nc.sync.dma_start(ph0[:], phi_d[0])
            nc.sync.dma_start(po0[:], pod_d[0])
            nc.sync.dma_start(ph1[:], phi_d[1])
            nc.sync.dma_start(po1[:], pod_d[1])
            nc.sync.dma_start(zcat_t[:], zcat_d[:])
            nc.sync.dma_start(coef_t[:], coef_d[:])
            nc.sync.dma_start(om_t[:], om_d[:])
            nc.vector.tensor_scalar_mul(w_t[:], zcat_t[:], coef_t[:, 0:1])

            y0 = featp.tile([128, L], F32, tag="y0")
            y1 = featp.tile([128, L], F32, tag="y1")
            nc.vector.tensor_mul(y0[:], ph0[:], po0[:])
            nc.vector.tensor_mul(y1[:], ph1[:], po1[:])

            # latent output: interleave y0/y1 -> (n, 2) layout
            il = featp.tile([128, 2 * L], F32, tag="il")
            ilv = il[:].rearrange("p (j s) -> p j s", s=2)
            nc.vector.tensor_copy(ilv[:, :, 0], y0[:])
            nc.vector.tensor_copy(ilv[:, :, 1], y1[:])
            nc.sync.dma_start(lat_d[:], il[:])

            names = [
                "y0sq", "y0y1", "s0", "c0", "t0",
                "y1sq", "s1", "c1", "t1",
            ]
            ft = {nm: featp.tile([128, L], F32R, tag=nm, name=nm) for nm in names}
            nc.vector.tensor_mul(ft["y0sq"][:], y0[:], y0[:])
            nc.vector.tensor_mul(ft["y1sq"][:], y1[:], y1[:])
            nc.vector.tensor_mul(ft["y0y1"][:], y0[:], y1[:])

            # Trig features. The Sin activation is only valid on [-pi, pi],
            # so range-reduce a = omega*y first: k = round(a/2pi) (fp32
            # magic-number rounding), r = a - k*2pi, then wrap with the sin/
            # cos shift back into [-pi, pi] (rotary recipe).
            TWO_PI = float(2.0 * np.pi)
            PI = float(np.pi)
            MAGIC = float(1.5 * 2.0**23)
            mul_op = mybir.AluOpType.mult
            add_op = mybir.AluOpType.add

            def trig(dst, y_t, om_col, shift, idx):
                a = featp.tile([128, L], F32, tag=f"tga{idx}", name=f"tga{idx}")
                kf = featp.tile([128, L], F32, tag=f"tgk{idx}", name=f"tgk{idx}")
                rr = featp.tile([128, L], F32, tag=f"tgr{idx}", name=f"tgr{idx}")
                nc.vector.tensor_scalar_mul(a[:], y_t[:], om_t[:, om_col : om_col + 1])
                nc.vector.tensor_scalar(
                    kf[:], a[:], 1.0 / TWO_PI, MAGIC, mul_op, add_op
                )
                nc.vector.tensor_scalar_add(kf[:], kf[:], -MAGIC)
                nc.vector.scalar_tensor_tensor(
                    rr[:], kf[:], -TWO_PI, a[:], mul_op, add_op
                )
                nc.vector.add_range_wrap(a[:], rr[:], shift, PI, TWO_PI)
                nc.scalar.activation(dst[:], a[:], Sin)

            trig(ft["s0"], y0, 0, 0.0, 0)
            trig(ft["c0"], y0, 1, HALF_PI, 1)
            trig(ft["s1"], y1, 3, 0.0, 2)
            trig(ft["c1"], y1, 4, HALF_PI, 3)
            nc.scalar.activation(ft["t0"][:], y0[:], Tanh, scale=om_t[:, 2:3])
            nc.scalar.activation(ft["t1"][:], y1[:], Tanh, scale=om_t[:, 5:6])

            # ---- F matrix (12, NL): row k <- reshape of (128, L) feature ----
            # F matrix split in 4 quarters along n, quarter q at
            # partitions [32q, 32q+12): fq[32q+k, j] = F[k, 3136q + j]
            fbig = fbigp.tile([128, QW], F32R, tag="fbig")
            ones_f = featp.tile([128, L], F32, tag="ones_f")
            ones_r = featp.tile([128, L], F32R, tag="ones_r")
            nc.vector.memset(ones_f[:], 1.0)
            nc.vector.tensor_copy(ones_r[:], ones_f[:])
            y0r = featp.tile([128, L], F32R, tag="y0r")
            y1r = featp.tile([128, L], F32R, tag="y1r")
            nc.vector.tensor_copy(y0r[:], y0[:])
            nc.vector.tensor_copy(y1r[:], y1[:])
            row_srcs = [
                ones_r,
                y0r, ft["y0sq"], ft["y0y1"], ft["s0"], ft["c0"], ft["t0"],
                y1r, ft["y1sq"], ft["s1"], ft["c1"], ft["t1"],
            ]
            # issue in feature-readiness order (cheap poly rows first, trig
            # last); quarter-0 pieces first so the first matmuls unblock
            # before quarters 1-3 land.
            ready_order = [0, 1, 7, 2, 8, 3, 6, 11, 4, 5, 9, 10]
            for k in ready_order:
                srct = row_srcs[k]
                nc.sync.dma_start(fbig[k : k + 1, :QW], srct[0:32, :])
            for k in ready_order:
                srct = row_srcs[k]
                nc.sync.dma_start(fbig[32 + k :: 32, :], srct[32:128, :])

            # ---- main loop: out(m, n) = W^T @ F ----
            # Four concurrent K=12 matmuls per chunk, one per PE row group
            # (tile_position auto-derived from the base partition 32q).
            for ti, (m0, mt) in enumerate(M_TILES):
                band = bandp.tile([128, NL], F32, tag="band")
                if ti == 0:
                    # pair-major: 2-way PE packing; each quarter-pair's 3.2MB
                    # streams out while the next pair computes
                    qc = [(q, c) for qp in range(2)
                          for c in range(NCHUNK) for q in (2 * qp, 2 * qp + 1)]
                else:
                    qc = [(q, c) for c in range(NCHUNK) for q in range(4)]
                for q, c in qc:
                    ps = psump.tile([128, NF], F32, tag="ps")
                    nc.tensor.matmul(
                        ps[:mt, :],
                        w_t[32 * q : 32 * q + 12, m0 : m0 + mt],
                        fbig[32 * q : 32 * q + 12, c * NF : (c + 1) * NF],
                        start=True,
                        stop=True,
                        tile_position=(32 * q, 0),
                    )
                    col = QW * q + NF * c
                    if (q + c) % 2 == 0:
                        nc.vector.tensor_copy(band[:mt, col : col + NF], ps[:mt, :])
                    else:
                        nc.scalar.copy(band[:mt, col : col + NF], ps[:mt, :])
                    if ti == 0 and c == NCHUNK - 1 and q % 2 == 1:
                        # quarters stream out in pairs as they complete
                        lo = QW * (q - 1)
                        hi = QW * (q + 1)
                        nc.sync.dma_start(
                            out_d[m0 : m0 + mt, lo:hi], band[:mt, lo:hi]
                        )
                if ti > 0:
                    nc.sync.dma_start(out_d[m0 : m0 + mt, :], band[:mt, :])

    nc.compile()
    return nc


def _get_compiled():
    global _COMPILED
    if _COMPILED is None:
        _COMPILED = _build()
    return _COMPILED


def _make_in_maps(inputs):
    phi = np.ascontiguousarray(np.asarray(inputs["phi"], dtype=np.float32))
    pod = np.asarray(inputs["POD_modes"], dtype=np.float32)
    podT = np.ascontiguousarray(pod.T)  # (2, N)
    c = np.asarray(inputs["c_coef"], dtype=np.float32)
    zv = np.asarray(inputs["z_values"], dtype=np.float32)
    zs = np.asarray(inputs["zsin"], dtype=np.float32)
    zc = np.asarray(inputs["zcos"], dtype=np.float32)
    zt = np.asarray(inputs["ztanh"], dtype=np.float32)
    sc = np.asarray(inputs["sin_coef"], dtype=np.float32)
    cc = np.asarray(inputs["cos_coef"], dtype=np.float32)
    tc_ = np.asarray(inputs["tanh_coef"], dtype=np.float32)
    om = np.asarray(inputs["omega"], dtype=np.float32)

    zcat12 = np.stack(
        [zv[0], zv[1], zv[3], zv[4], zs[0], zc[0], zt[0],
         zv[2], zv[5], zs[1], zc[1], zt[1]], axis=0
    ).astype(np.float32)
    coef12 = np.array(
        [c[0], c[1], c[3], c[4], sc[0], cc[0], tc_[0],
         c[2], c[5], sc[1], cc[1], tc_[1]], dtype=np.float32
    ).reshape(12, 1)
    zcat = np.zeros((128, M), np.float32)
    coef = np.zeros((128, 1), np.float32)
    for q in range(4):
        zcat[32 * q : 32 * q + 12] = zcat12
        coef[32 * q : 32 * q + 12] = coef12
    om_rep = np.ascontiguousarray(np.tile(om.reshape(1, 6), (128, 1)))

    in_maps = []
    for ci in range(N_CORES):
        lo = ci * NLOC
        phi_l = np.zeros((2, NL), np.float32)
        pod_l = np.zeros((2, NL), np.float32)
        phi_l[:, :NLOC] = phi[:, lo : lo + NLOC]
        pod_l[:, :NLOC] = podT[:, lo : lo + NLOC]
        in_maps.append(
            {
                "phi_l": phi_l.reshape(2, 128, L),
                "pod_l": pod_l.reshape(2, 128, L),
                "zcat": zcat,
                "coef": coef,
                "om": om_rep,
            }
        )
    return in_maps


def _run(inputs, trace=False):
    nc = _get_compiled()
    in_maps = _make_in_maps(inputs)
    res = run_bass_kernel_spmd(nc, in_maps, core_ids=list(range(N_CORES)), trace=trace)

    out = np.empty((M, N), np.float32)
    lat = np.empty((N, 2), np.float32)
    for ci in range(N_CORES):
        lo = ci * NLOC
        out[:, lo : lo + NLOC] = res.results[ci]["out_l"][:, :NLOC]
        lat[lo : lo + NLOC] = res.results[ci]["lat_l"].reshape(-1, 2)[:NLOC]
    zv = np.asarray(inputs["z_values"], dtype=np.float32)
    return (out, lat, zv), res


def kernel(**inputs):
    (out, lat, zv), _ = _run(inputs, trace=False)
    return out, lat, zv


# revision 23
# speedup vs baseline: 1.0577x; 1.0538x over previous
"""Trainium2 Bass kernel for nn_DESMO (SINDy/POD reconstruction).

Math (per reference):
  y0 = phi[0] * POD[:,0],  y1 = phi[1] * POD[:,1]            (n,)
  out(m, n) = W^T @ F
    F (12, n) = [1, y0, y0^2, y0*y1, sin(w0*y0), cos(w1*y0), tanh(w2*y0),
                 y1, y1^2, sin(w3*y1), cos(w4*y1), tanh(w5*y1)]
    W (12, m) = [c0*z0, c1*z1, c3*z3, c4*z4, sc0*zsin0, cc0*zcos0, tc0*ztanh0,
                 c2*z2, c5*z5, sc1*zsin1, cc1*zcos1, tc1*ztanh1]
  latent (n, 2) = [y0, y1]
  z_values passthrough.

Sharding: n (spatial) split evenly across 8 NeuronCores; z rows / coefs
replicated; no cross-core communication.
"""

import numpy as np

from concourse import bacc, bass, mybir, tile
from concourse.bass_utils import run_bass_kernel_spmd

F32 = mybir.dt.float32
F32R = mybir.dt.float32r

N_CORES = 8
N, M, R = 100000, 1000, 2
NLOC = N // N_CORES          # 12500 spatial points per core
L = 98                       # free-dim elems per partition chunk
NL = 128 * L                 # 12544 = padded local n
NF = 448                     # matmul free-dim chunk (fits one PSUM bank)
QW = NL // 4                 # 3136 columns per PE row-group quarter
NCHUNK = QW // NF            # 7 chunks per quarter

# m tiles: 7 x 128 + 104
M_TILES = [(i * 128, min(128, M - i * 128)) for i in range((M + 127) // 128)]

_COMPILED = None


def _build():
    nc = bacc.Bacc(
        "TRN2",
        target_bir_lowering=False,
        debug=False,
        enable_asserts=True,
        num_devices=N_CORES,
    )

    phi_d = nc.dram_tensor("phi_l", [2, 128, L], F32, kind="ExternalInput")
    pod_d = nc.dram_tensor("pod_l", [2, 128, L], F32, kind="ExternalInput")
    # zcat/coef carry 4 replicas at partition offsets 0/32/64/96 so four
    # K=12 matmuls can run concurrently in the PE's four 32-row groups.
    zcat_d = nc.dram_tensor("zcat", [128, M], F32, kind="ExternalInput")
    coef_d = nc.dram_tensor("coef", [128, 1], F32, kind="ExternalInput")
    om_d = nc.dram_tensor("om", [128, 6], F32, kind="ExternalInput")

    out_d = nc.dram_tensor("out_l", [M, NL], F32, kind="ExternalOutput")
    lat_d = nc.dram_tensor("lat_l", [128, 2 * L], F32, kind="ExternalOutput")

    HALF_PI = float(np.pi / 2.0)
    Sin = mybir.ActivationFunctionType.Sin
    Tanh = mybir.ActivationFunctionType.Tanh

    with tile.TileContext(nc) as tc:
        with (
            tc.tile_pool(name="const", bufs=1) as constp,
            tc.tile_pool(name="feat", bufs=1) as featp,
            tc.tile_pool(name="fbig", bufs=1) as fbigp,
            tc.tile_pool(name="band", bufs=3) as bandp,
            tc.tile_pool(name="psum", bufs=8, space="PSUM") as psump,
        ):
            # ---- W = coef * zcat, replicated in 4 row groups ----
            zcat_t = constp.tile([128, M], F32, tag="zcat")
            coef_t = constp.tile([128, 1], F32, tag="coef")
            om_t = constp.tile([128, 6], F32, tag="om")
            w_t = constp.tile([128, M], F32R, tag="w")
            # ---- features, (128, L) p-major layout (inputs first) ----
            ph0 = featp.tile([128, L], F32, tag="ph0")
            ph1 = featp.tile([128, L], F32, tag="ph1")
            po0 = featp.tile([128, L], F32, tag="po0")
            po1 = featp.tile([128, L], F32, tag="po1")
            nc.sync.dma_start(ph0[:], phi_d[0])
            nc.sync.dma_start(po0[:], pod_d[0])
            nc.sync.dma_start(ph1[:], phi_d[1])
            nc.sync.dma_start(po1[:], pod_d[1])
            nc.sync.dma_start(zcat_t[:], zcat_d[:])
            nc.sync.dma_start(coef_t[:], coef_d[:])
            nc.sync.dma_start(om_t[:], om_d[:])
            nc.vector.tensor_scalar_mul(w_t[:], zcat_t[:], coef_t[:, 0:1])

            y0 = featp.tile([128, L], F32, tag="y0")
            y1 = featp.tile([128, L], F32, tag="y1")
            nc.vector.tensor_mul(y0[:], ph0[:], po0[:])
            nc.vector.tensor_mul(y1[:], ph1[:], po1[:])

            # latent output: interleave y0/y1 -> (n, 2) layout
            il = featp.tile([128, 2 * L], F32, tag="il")
            ilv = il[:].rearrange("p (j s) -> p j s", s=2)
            nc.vector.tensor_copy(ilv[:, :, 0], y0[:])
            nc.vector.tensor_copy(ilv[:, :, 1], y1[:])
            nc.sync.dma_start(lat_d[:], il[:])

            names = [
                "y0sq", "y0y1", "s0", "c0", "t0",
                "y1sq", "s1", "c1", "t1",
            ]
            ft = {nm: featp.tile([128, L], F32R, tag=nm, name=nm) for nm in names}
            nc.vector.tensor_mul(ft["y0sq"][:], y0[:], y0[:])
            nc.vector.tensor_mul(ft["y1sq"][:], y1[:], y1[:])
            nc.vector.tensor_mul(ft["y0y1"][:], y0[:], y1[:])

            # Trig features. The Sin activation is only valid on [-pi, pi],
            # so range-reduce a = omega*y first: k = round(a/2pi) (fp32
            # magic-number rounding), r = a - k*2pi, then wrap with the sin/
            # cos shift back into [-pi, pi] (rotary recipe).
            TWO_PI = float(2.0 * np.pi)
            PI = float(np.pi)
            MAGIC = float(1.5 * 2.0**23)
            mul_op = mybir.AluOpType.mult
            add_op = mybir.AluOpType.add

            def trig(dst, y_t, om_col, shift, idx):
                a = featp.tile([128, L], F32, tag=f"tga{idx}", name=f"tga{idx}")
                kf = featp.tile([128, L], F32, tag=f"tgk{idx}", name=f"tgk{idx}")
                rr = featp.tile([128, L], F32, tag=f"tgr{idx}", name=f"tgr{idx}")
                nc.vector.tensor_scalar_mul(a[:], y_t[:], om_t[:, om_col : om_col + 1])
                nc.vector.tensor_scalar(
                    kf[:], a[:], 1.0 / TWO_PI, MAGIC, mul_op, add_op
                )
                nc.vector.tensor_scalar_add(kf[:], kf[:], -MAGIC)
                nc.vector.scalar_tensor_tensor(
                    rr[:], kf[:], -TWO_PI, a[:], mul_op, add_op
                )
                nc.vector.add_range_wrap(a[:], rr[:], shift, PI, TWO_PI)
                nc.scalar.activation(dst[:], a[:], Sin)

            trig(ft["s0"], y0, 0, 0.0, 0)
            trig(ft["c0"], y0, 1, HALF_PI, 1)
            trig(ft["s1"], y1, 3, 0.0, 2)
            trig(ft["c1"], y1, 4, HALF_PI, 3)
            nc.scalar.activation(ft["t0"][:], y0[:], Tanh, scale=om_t[:, 2:3])
            nc.scalar.activation(ft["t1"][:], y1[:], Tanh, scale=om_t[:, 5:6])

            # ---- F matrix (12, NL): row k <- reshape of (128, L) feature ----
            # F matrix split in 4 quarters along n, quarter q at
            # partitions [32q, 32q+12): fq[32q+k, j] = F[k, 3136q + j]
            fbig = fbigp.tile([128, QW], F32R, tag="fbig")
            ones_f = featp.tile([128, L], F32, tag="ones_f")
            ones_r = featp.tile([128, L], F32R, tag="ones_r")
            nc.vector.memset(ones_f[:], 1.0)
            nc.vector.tensor_copy(ones_r[:], ones_f[:])
            y0r = featp.tile([128, L], F32R, tag="y0r")
            y1r = featp.tile([128, L], F32R, tag="y1r")
            nc.vector.tensor_copy(y0r[:], y0[:])
            nc.vector.tensor_copy(y1r[:], y1[:])
            row_srcs = [
                ones_r,
                y0r, ft["y0sq"], ft["y0y1"], ft["s0"], ft["c0"], ft["t0"],
                y1r, ft["y1sq"], ft["s1"], ft["c1"], ft["t1"],
            ]
            # issue in feature-readiness order (cheap poly rows first, trig
            # last); quarter-0 pieces first so the first matmuls unblock
            # before quarters 1-3 land.
            ready_order = [0, 1, 7, 2, 8, 3, 6, 11, 4, 5, 9, 10]
            for k in ready_order:
                srct = row_srcs[k]
                nc.sync.dma_start(fbig[k : k + 1, :QW], srct[0:32, :])
            for k in ready_order:
                srct = row_srcs[k]
                nc.scalar.dma_start(fbig[32 + k :: 32, :], srct[32:128, :])

            # ---- main loop: out(m, n) = W^T @ F ----
            # Four concurrent K=12 matmuls per chunk, one per PE row group
            # (tile_position auto-derived from the base partition 32q).
            for ti, (m0, mt) in enumerate(M_TILES):
                band = bandp.tile([128, NL], F32, tag="band")
                if ti == 0:
                    # pair-major: 2-way PE packing; each quarter-pair's 3.2MB
                    # streams out while the next pair computes
                    qc = [(q, c) for q in range(4) for c in range(NCHUNK)]
                else:
                    qc = [(q, c) for c in range(NCHUNK) for q in range(4)]
                for q, c in qc:
                    ps = psump.tile([128, NF], F32, tag="ps")
                    nc.tensor.matmul(
                        ps[:mt, :],
                        w_t[32 * q : 32 * q + 12, m0 : m0 + mt],
                        fbig[32 * q : 32 * q + 12, c * NF : (c + 1) * NF],
                        start=True,
                        stop=True,
                        tile_position=(32 * q, 0),
                    )
                    col = QW * q + NF * c
                    if (q + c) % 2 == 0:
                        nc.vector.tensor_copy(band[:mt, col : col + NF], ps[:mt, :])
                    else:
                        nc.scalar.copy(band[:mt, col : col + NF], ps[:mt, :])
                    if ti == 0 and q == 0 and c == 3:
                        # band 0 quarter 0: stream the first 4 chunks ASAP
                        nc.sync.dma_start(
                            out_d[m0 : m0 + mt, : 4 * NF], band[:mt, : 4 * NF]
                        )
                    elif ti == 0 and q == 0 and c == NCHUNK - 1:
                        nc.sync.dma_start(
                            out_d[m0 : m0 + mt, 4 * NF : QW],
                            band[:mt, 4 * NF : QW],
                        )
                    elif ti <= 1 and c == NCHUNK - 1:
                        # each quarter streams out as soon as it completes
                        nc.sync.dma_start(
                            out_d[m0 : m0 + mt, QW * q : QW * (q + 1)],
                            band[:mt, QW * q : QW * (q + 1)],
                        )
                if ti > 1:
                    nc.sync.dma_start(out_d[m0 : m0 + mt, :], band[:mt, :])

    nc.compile()
    return nc


def _get_compiled():
    global _COMPILED
    if _COMPILED is None:
        _COMPILED = _build()
    return _COMPILED


def _make_in_maps(inputs):
    phi = np.ascontiguousarray(np.asarray(inputs["phi"], dtype=np.float32))
    pod = np.asarray(inputs["POD_modes"], dtype=np.float32)
    podT = np.ascontiguousarray(pod.T)  # (2, N)
    c = np.asarray(inputs["c_coef"], dtype=np.float32)
    zv = np.asarray(inputs["z_values"], dtype=np.float32)
    zs = np.asarray(inputs["zsin"], dtype=np.float32)
    zc = np.asarray(inputs["zcos"], dtype=np.float32)
    zt = np.asarray(inputs["ztanh"], dtype=np.float32)
    sc = np.asarray(inputs["sin_coef"], dtype=np.float32)
    cc = np.asarray(inputs["cos_coef"], dtype=np.float32)
    tc_ = np.asarray(inputs["tanh_coef"], dtype=np.float32)
    om = np.asarray(inputs["omega"], dtype=np.float32)

    zcat12 = np.stack(
        [zv[0], zv[1], zv[3], zv[4], zs[0], zc[0], zt[0],
         zv[2], zv[5], zs[1], zc[1], zt[1]], axis=0
    ).astype(np.float32)
    coef12 = np.array(
        [c[0], c[1], c[3], c[4], sc[0], cc[0], tc_[0],
         c[2], c[5], sc[1], cc[1], tc_[1]], dtype=np.float32
    ).reshape(12, 1)
    zcat = np.zeros((128, M), np.float32)
    coef = np.zeros((128, 1), np.float32)
    for q in range(4):
        zcat[32 * q : 32 * q + 12] = zcat12
        coef[32 * q : 32 * q + 12] = coef12
    om_rep = np.ascontiguousarray(np.tile(om.reshape(1, 6), (128, 1)))

    in_maps = []
    for ci in range(N_CORES):
        lo = ci * NLOC
        phi_l = np.zeros((2, NL), np.float32)
        pod_l = np.zeros((2, NL), np.float32)
        phi_l[:, :NLOC] = phi[:, lo : lo + NLOC]
        pod_l[:, :NLOC] = podT[:, lo : lo + NLOC]
        in_maps.append(
            {
                "phi_l": phi_l.reshape(2, 128, L),
                "pod_l": pod_l.reshape(2, 128, L),
                "zcat": zcat,
                "coef": coef,
                "om": om_rep,
            }
        )
    return in_maps


def _run(inputs, trace=False):
    nc = _get_compiled()
    in_maps = _make_in_maps(inputs)
    res = run_bass_kernel_spmd(nc, in_maps, core_ids=list(range(N_CORES)), trace=trace)

    out = np.empty((M, N), np.float32)
    lat = np.empty((N, 2), np.float32)
    for ci in range(N_CORES):
        lo = ci * NLOC
        out[:, lo : lo + NLOC] = res.results[ci]["out_l"][:, :NLOC]
        lat[lo : lo + NLOC] = res.results[ci]["lat_l"].reshape(-1, 2)[:NLOC]
    zv = np.asarray(inputs["z_values"], dtype=np.float32)
    return (out, lat, zv), res


def kernel(**inputs):
    (out, lat, zv), _ = _run(inputs, trace=False)
    return out, lat, zv


# revision 26
# speedup vs baseline: 1.0606x; 1.0028x over previous
"""Trainium2 Bass kernel for nn_DESMO (SINDy/POD reconstruction).

Math (per reference):
  y0 = phi[0] * POD[:,0],  y1 = phi[1] * POD[:,1]            (n,)
  out(m, n) = W^T @ F
    F (12, n) = [1, y0, y0^2, y0*y1, sin(w0*y0), cos(w1*y0), tanh(w2*y0),
                 y1, y1^2, sin(w3*y1), cos(w4*y1), tanh(w5*y1)]
    W (12, m) = [c0*z0, c1*z1, c3*z3, c4*z4, sc0*zsin0, cc0*zcos0, tc0*ztanh0,
                 c2*z2, c5*z5, sc1*zsin1, cc1*zcos1, tc1*ztanh1]
  latent (n, 2) = [y0, y1]
  z_values passthrough.

Sharding: n (spatial) split evenly across 8 NeuronCores; z rows / coefs
replicated; no cross-core communication.
"""

import numpy as np

from concourse import bacc, mybir, tile
from concourse.bass_utils import run_bass_kernel_spmd

F32 = mybir.dt.float32
F32R = mybir.dt.float32r

N_CORES = 8
N, M, R = 100000, 1000, 2
NLOC = N // N_CORES          # 12500 spatial points per core
L = 98                       # free-dim elems per partition chunk
NL = 128 * L                 # 12544 = padded local n
NF = 448                     # matmul free-dim chunk (fits one PSUM bank)
QW = NL // 4                 # 3136 columns per PE row-group quarter
NCHUNK = QW // NF            # 7 chunks per quarter

# m tiles: 7 x 128 + 104
M_TILES = [(i * 128, min(128, M - i * 128)) for i in range((M + 127) // 128)]

_COMPILED = None


def _build():
    nc = bacc.Bacc(
        "TRN2",
        target_bir_lowering=False,
        debug=False,
        enable_asserts=True,
        num_devices=N_CORES,
    )

    phi_d = nc.dram_tensor("phi_l", [2, 128, L], F32, kind="ExternalInput")
    pod_d = nc.dram_tensor("pod_l", [2, 128, L], F32, kind="ExternalInput")
    # zcat/coef carry 4 replicas at partition offsets 0/32/64/96 so four
    # K=12 matmuls can run concurrently in the PE's four 32-row groups.
    zcat_d = nc.dram_tensor("zcat", [128, M], F32, kind="ExternalInput")
    coef_d = nc.dram_tensor("coef", [128, 1], F32, kind="ExternalInput")
    om_d = nc.dram_tensor("om", [128, 6], F32, kind="ExternalInput")

    out_d = nc.dram_tensor("out_l", [M, NL], F32, kind="ExternalOutput")
    lat_d = nc.dram_tensor("lat_l", [128, 2 * L], F32, kind="ExternalOutput")

    HALF_PI = float(np.pi / 2.0)
    Sin = mybir.ActivationFunctionType.Sin
    Tanh = mybir.ActivationFunctionType.Tanh

    with tile.TileContext(nc) as tc:
        with (
            tc.tile_pool(name="const", bufs=1) as constp,
            tc.tile_pool(name="feat", bufs=1) as featp,
            tc.tile_pool(name="fbig", bufs=1) as fbigp,
            tc.tile_pool(name="band", bufs=3) as bandp,
            tc.tile_pool(name="psum", bufs=8, space="PSUM") as psump,
        ):
            # ---- W = coef * zcat, replicated in 4 row groups ----
            zcat_t = constp.tile([128, M], F32, tag="zcat")
            coef_t = constp.tile([128, 1], F32, tag="coef")
            om_t = constp.tile([128, 6], F32, tag="om")
            w_t = constp.tile([128, M], F32R, tag="w")
            # ---- features, (128, L) p-major layout (inputs first) ----
            ph0 = featp.tile([128, L], F32, tag="ph0")
            ph1 = featp.tile([128, L], F32, tag="ph1")
            po0 = featp.tile([128, L], F32, tag="po0")
            po1 = featp.tile([128, L], F32, tag="po1")
            nc.sync.dma_start(ph0[:], phi_d[0])
            nc.sync.dma_start(po0[:], pod_d[0])
            nc.sync.dma_start(ph1[:], phi_d[1])
            nc.sync.dma_start(po1[:], pod_d[1])
            nc.sync.dma_start(zcat_t[:], zcat_d[:])
            nc.sync.dma_start(coef_t[:], coef_d[:])
            nc.sync.dma_start(om_t[:], om_d[:])
            nc.vector.tensor_scalar_mul(w_t[:], zcat_t[:], coef_t[:, 0:1])

            y0 = featp.tile([128, L], F32, tag="y0")
            y1 = featp.tile([128, L], F32, tag="y1")
            nc.vector.tensor_mul(y0[:], ph0[:], po0[:])
            nc.vector.tensor_mul(y1[:], ph1[:], po1[:])

            # latent output: interleave y0/y1 -> (n, 2) layout
            il = featp.tile([128, 2 * L], F32, tag="il")
            ilv = il[:].rearrange("p (j s) -> p j s", s=2)
            nc.vector.tensor_copy(ilv[:, :, 0], y0[:])
            nc.vector.tensor_copy(ilv[:, :, 1], y1[:])
            nc.sync.dma_start(lat_d[:], il[:])

            names = [
                "y0sq", "y0y1", "s0", "c0", "t0",
                "y1sq", "s1", "c1", "t1",
            ]
            ft = {nm: featp.tile([128, L], F32R, tag=nm, name=nm) for nm in names}
            nc.vector.tensor_mul(ft["y0sq"][:], y0[:], y0[:])
            nc.vector.tensor_mul(ft["y1sq"][:], y1[:], y1[:])
            nc.vector.tensor_mul(ft["y0y1"][:], y0[:], y1[:])

            # Trig features. The Sin activation is only valid on [-pi, pi],
            # so range-reduce a = omega*y first: k = round(a/2pi) (fp32
            # magic-number rounding), r = a - k*2pi, then wrap with the sin/
            # cos shift back into [-pi, pi] (rotary recipe).
            TWO_PI = float(2.0 * np.pi)
            PI = float(np.pi)
            MAGIC = float(1.5 * 2.0**23)
            mul_op = mybir.AluOpType.mult
            add_op = mybir.AluOpType.add

            def trig(dst, y_t, om_col, shift, idx):
                a = featp.tile([128, L], F32, tag=f"tga{idx}", name=f"tga{idx}")
                kf = featp.tile([128, L], F32, tag=f"tgk{idx}", name=f"tgk{idx}")
                rr = featp.tile([128, L], F32, tag=f"tgr{idx}", name=f"tgr{idx}")
                nc.vector.tensor_scalar_mul(a[:], y_t[:], om_t[:, om_col : om_col + 1])
                nc.vector.tensor_scalar(
                    kf[:], a[:], 1.0 / TWO_PI, MAGIC, mul_op, add_op
                )
                nc.vector.tensor_scalar_add(kf[:], kf[:], -MAGIC)
                nc.vector.scalar_tensor_tensor(
                    rr[:], kf[:], -TWO_PI, a[:], mul_op, add_op
                )
                nc.vector.add_range_wrap(a[:], rr[:], shift, PI, TWO_PI)
                nc.scalar.activation(dst[:], a[:], Sin)

            trig(ft["s0"], y0, 0, 0.0, 0)
            trig(ft["c0"], y0, 1, HALF_PI, 1)
            trig(ft["s1"], y1, 3, 0.0, 2)
            trig(ft["c1"], y1, 4, HALF_PI, 3)
            nc.scalar.activation(ft["t0"][:], y0[:], Tanh, scale=om_t[:, 2:3])
            nc.scalar.activation(ft["t1"][:], y1[:], Tanh, scale=om_t[:, 5:6])

            # ---- F matrix (12, NL): row k <- reshape of (128, L) feature ----
            # F matrix split in 4 quarters along n, quarter q at
            # partitions [32q, 32q+12): fq[32q+k, j] = F[k, 3136q + j]
            fbig = fbigp.tile([128, QW], F32R, tag="fbig")
            ones_f = featp.tile([128, L], F32, tag="ones_f")
            ones_r = featp.tile([128, L], F32R, tag="ones_r")
            nc.vector.memset(ones_f[:], 1.0)
            nc.vector.tensor_copy(ones_r[:], ones_f[:])
            y0r = featp.tile([128, L], F32R, tag="y0r")
            y1r = featp.tile([128, L], F32R, tag="y1r")
            nc.vector.tensor_copy(y0r[:], y0[:])
            nc.vector.tensor_copy(y1r[:], y1[:])
            row_srcs = [
                ones_r,
                y0r, ft["y0sq"], ft["y0y1"], ft["s0"], ft["c0"], ft["t0"],
                y1r, ft["y1sq"], ft["s1"], ft["c1"], ft["t1"],
            ]
            # issue in feature-readiness order (cheap poly rows first, trig
            # last); quarter-0 pieces first so the first matmuls unblock
            # before quarters 1-3 land.
            ready_order = [0, 1, 7, 2, 8, 3, 6, 11, 4, 5, 9, 10]
            for k in ready_order:
                srct = row_srcs[k]
                nc.sync.dma_start(fbig[k : k + 1, :QW], srct[0:32, :])
            for k in ready_order:
                srct = row_srcs[k]
                nc.scalar.dma_start(fbig[32 + k :: 32, :], srct[32:128, :])

            # ---- main loop: out(m, n) = W^T @ F ----
            # Four concurrent K=12 matmuls per chunk, one per PE row group
            # (tile_position auto-derived from the base partition 32q).
            for ti, (m0, mt) in enumerate(M_TILES):
                band = bandp.tile([128, NL], F32, tag="band")
                if ti == 0:
                    # quarter-major: quarter q's slice can stream out while
                    # the next quarter computes (fills the DMA ramp early)
                    qc = [(q, c) for q in range(4) for c in range(NCHUNK)]
                else:
                    qc = [(q, c) for c in range(NCHUNK) for q in range(4)]
                for q, c in qc:
                    ps = psump.tile([128, NF], F32, tag="ps")
                    nc.tensor.matmul(
                        ps[:mt, :],
                        w_t[32 * q : 32 * q + 12, m0 : m0 + mt],
                        fbig[32 * q : 32 * q + 12, c * NF : (c + 1) * NF],
                        start=True,
                        stop=True,
                        tile_position=(32 * q, 0),
                    )
                    col = QW * q + NF * c
                    if (q + c) % 2 == 0:
                        nc.vector.tensor_copy(band[:mt, col : col + NF], ps[:mt, :])
                    else:
                        nc.scalar.copy(band[:mt, col : col + NF], ps[:mt, :])
                    if ti == 0 and q == 0 and c == 3:
                        # band 0 quarter 0: stream the first 4 chunks ASAP
                        nc.sync.dma_start(
                            out_d[m0 : m0 + mt, : 4 * NF], band[:mt, : 4 * NF]
                        )
                    elif ti == 0 and q == 0 and c == NCHUNK - 1:
                        nc.sync.dma_start(
                            out_d[m0 : m0 + mt, 4 * NF : QW],
                            band[:mt, 4 * NF : QW],
                        )
                    elif ti <= 1 and c == NCHUNK - 1:
                        # each quarter streams out as soon as it completes
                        nc.sync.dma_start(
                            out_d[m0 : m0 + mt, QW * q : QW * (q + 1)],
                            band[:mt, QW * q : QW * (q + 1)],
                        )
                if ti > 1:
                    nc.sync.dma_start(out_d[m0 : m0 + mt, :], band[:mt, :])

    nc.compile()
    return nc


def _get_compiled():
    global _COMPILED
    if _COMPILED is None:
        _COMPILED = _build()
    return _COMPILED


def _make_in_maps(inputs):
    phi = np.ascontiguousarray(np.asarray(inputs["phi"], dtype=np.float32))
    pod = np.asarray(inputs["POD_modes"], dtype=np.float32)
    podT = np.ascontiguousarray(pod.T)  # (2, N)
    c = np.asarray(inputs["c_coef"], dtype=np.float32)
    zv = np.asarray(inputs["z_values"], dtype=np.float32)
    zs = np.asarray(inputs["zsin"], dtype=np.float32)
    zc = np.asarray(inputs["zcos"], dtype=np.float32)
    zt = np.asarray(inputs["ztanh"], dtype=np.float32)
    sc = np.asarray(inputs["sin_coef"], dtype=np.float32)
    cc = np.asarray(inputs["cos_coef"], dtype=np.float32)
    tc_ = np.asarray(inputs["tanh_coef"], dtype=np.float32)
    om = np.asarray(inputs["omega"], dtype=np.float32)

    zcat12 = np.stack(
        [zv[0], zv[1], zv[3], zv[4], zs[0], zc[0], zt[0],
         zv[2], zv[5], zs[1], zc[1], zt[1]], axis=0
    ).astype(np.float32)
    coef12 = np.array(
        [c[0], c[1], c[3], c[4], sc[0], cc[0], tc_[0],
         c[2], c[5], sc[1], cc[1], tc_[1]], dtype=np.float32
    ).reshape(12, 1)
    zcat = np.zeros((128, M), np.float32)
    coef = np.zeros((128, 1), np.float32)
    for q in range(4):
        zcat[32 * q : 32 * q + 12] = zcat12
        coef[32 * q : 32 * q + 12] = coef12
    om_rep = np.ascontiguousarray(np.tile(om.reshape(1, 6), (128, 1)))

    in_maps = []
    for ci in range(N_CORES):
        lo = ci * NLOC
        phi_l = np.zeros((2, NL), np.float32)
        pod_l = np.zeros((2, NL), np.float32)
        phi_l[:, :NLOC] = phi[:, lo : lo + NLOC]
        pod_l[:, :NLOC] = podT[:, lo : lo + NLOC]
        in_maps.append(
            {
                "phi_l": phi_l.reshape(2, 128, L),
                "pod_l": pod_l.reshape(2, 128, L),
                "zcat": zcat,
                "coef": coef,
                "om": om_rep,
            }
        )
    return in_maps


def _run(inputs, trace=False):
    nc = _get_compiled()
    in_maps = _make_in_maps(inputs)
    res = run_bass_kernel_spmd(nc, in_maps, core_ids=list(range(N_CORES)), trace=trace)

    out = np.empty((M, N), np.float32)
    lat = np.empty((N, 2), np.float32)
    for ci in range(N_CORES):
        lo = ci * NLOC
        out[:, lo : lo + NLOC] = res.results[ci]["out_l"][:, :NLOC]
        lat[lo : lo + NLOC] = res.results[ci]["lat_l"].reshape(-1, 2)[:NLOC]
    zv = np.asarray(inputs["z_values"], dtype=np.float32)
    return (out, lat, zv), res


def kernel(**inputs):
    (out, lat, zv), _ = _run(inputs, trace=False)
    return out, lat, zv
